# revision 11
# baseline (speedup 1.0000x reference)
"""Trainium2 Bass kernel for the MINE-style segment_reduce problem.

Computes, for the fixed problem size B=16384, L=512, HID=768, TRANS=128:

    mask   = target.astype(f32)                     # [B, L] of {0,1}
    counts = max(mask.sum(1), 1)
    lf     = (mask @ label_embed) / counts          # [B, HID]
    net(t) = MLP(concat(t @ W_text.T + b_text, lf @ W_label.T + b_label))
    out    = mean(softplus(net(text[perm]))) + mean(softplus(-net(text)))

Algebraic folding (exact in real arithmetic): the first two linear layers
collapse into

    h1 = relu(text @ A_t.T + (mask @ LW2) / counts + c0)
    A_t = W0[:, :T] @ W_text                        # [T, HID]
    LW2 = (label_embed @ W_label.T) @ W0[:, T:].T   # [L, T]
    c0  = b0 + W0[:, :T] @ b_text + W0[:, T:] @ b_label

so label_embed never reaches the device; the per-sample network is two
small matmuls + relu + softplus.

Sharding: data-parallel over B across 8 NeuronCores (2048 rows each).
negative_text = text[perm] is realized host-side as a per-shard gather.
Each core returns the partial softplus sum over its rows; the host adds
8 scalars and divides by B.

Device-side design (v2):
 - All per-tile bulk data (mask | text | neg-text, fp8, pair-interleaved
   for DoubleRow) is packed host-side into ONE dram blob per batch tile,
   so the whole input streams in with 5 large HWDGE DMAs on the sync
   ring (HWDGE descriptor-gen is ~600ns of serial SP time per DMA - the
   v1 kernel spent ~17us there across 27 DMAs).
 - Weight-stationary matmul ordering: each DoubleRow fp8 weight pair is
   loaded once per 2-tile supertile and streams 4 matmuls (2 tiles x
   2 streams), instead of paying the 213ns LDWEIGHTS per matmul.
 - v = mask @ LW2 lands in the same PSUM bank that the joint-stream text
   matmuls later accumulate into (WAR handled by Tile), and the e rows
   land in the h2 banks, so the whole pipeline fits in 8 PSUM banks
   with 2-supertile double buffering.
 - 1/counts is applied as a bf16 row broadcast (one HWDGE DMA) and a DVE
   multiply; c0 and the relu fold into one DVE tensor_scalar; b1 folds
   into the h2 relu on ACT.
 - softplus runs directly on the [1,512] e-rows on ACT with accum_out,
   with +-b2 as the activation bias, so there is no staging copy, no
   repack DMA, and no final cross-partition matmul (v1 spent ~7us in
   that serialized tail).
"""

import numpy as np
import ml_dtypes

B, L, HID, TRANS = 16384, 512, 768, 128
NCORES = 8
BS = B // NCORES          # 2048 rows per core
BT = 512                  # batch tile (free-dim columns per PSUM bank)
NT = BS // BT             # 4 tiles per core
HC = HID // 128           # 6 contraction chunks for text
LC = L // 128             # 4 contraction chunks for the mask
HP = HC // 2              # 3 DoubleRow pairs for text
LP = LC // 2              # 2 DoubleRow pairs for the mask

MT_B = LP * 2 * BT        # 2048 mask bytes per partition per tile
XT_B = HP * 2 * BT        # 3072 text bytes per partition per tile
TILE_B = MT_B + 2 * XT_B  # 8192 = mask | text | neg-text

BF16 = ml_dtypes.bfloat16
FP8 = ml_dtypes.float8_e4m3

_CACHE = {}


def _split_sync_waits(nc, mybir, maxw_default=1, maxw_drain=1):
    """Walrus in this container rejects too many sync-waits per instruction
    ("Too many sync wait commands"). Hoist excess waits onto NoOps that
    precede the instruction on the same engine."""
    for f in nc.m.functions:
        for bb in f.blocks:
            new = []
            for inst in bb.instructions:
                maxw = maxw_drain if type(inst).__name__ in ("InstDrain", "InstNoOp") else maxw_default
                si = inst.sync_info
                if si is not None and si.on_wait is not None and len(si.on_wait) > maxw:
                    waits = list(si.on_wait)
                    head, rest = waits[:-maxw], waits[-maxw:]
                    for k in range(0, len(head), maxw_drain):
                        nop = mybir.InstNoOp(name=f"{inst.name}-w{k}", ins=[], outs=[])
                        nop.engine = inst.engine
                        nop.sync_info = mybir.SyncInfo(
                            on_wait=head[k : k + maxw_drain], on_update=[]
                        )
                        new.append(nop)
                    inst.sync_info = mybir.SyncInfo(
                        on_wait=rest, on_update=list(si.on_update or [])
                    )
                new.append(inst)
            bb.instructions = new


N_WARM = 4


def _build():
    import concourse.bass as bass
    import concourse.mybir as mybir
    import concourse.tile as tile

    f32 = mybir.dt.float32
    bf16 = mybir.dt.bfloat16
    fp8 = mybir.dt.float8e4

    nc = bass.Bass("TRN2", target_bir_lowering=False, debug=False, num_devices=NCORES)

    blob_d = nc.declare_dram_parameter("blob", [128, NT, TILE_B], fp8, isOutput=False)
    wc8_d = nc.declare_dram_parameter("wc8", [128, (HC + LC) * TRANS], fp8, isOutput=False)
    wc16_d = nc.declare_dram_parameter("wc16", [128, TRANS + 2], bf16, isOutput=False)
    cf_d = nc.declare_dram_parameter("cf", [TRANS, 5], f32, isOutput=False)
    cb_d = nc.declare_dram_parameter("cbv", [1, BS], bf16, isOutput=False)
    out_d = nc.declare_dram_parameter("out", [1, 1], f32, isOutput=True)

    AF = mybir.ActivationFunctionType
    ALU = mybir.AluOpType
    DR = mybir.MatmulPerfMode.DoubleRow

    with tile.TileContext(nc) as tc:
        with (
            tc.tile_pool(name="const", bufs=1) as cpool,
            tc.tile_pool(name="blob", bufs=NT) as bpool,
            tc.tile_pool(name="vs", bufs=2) as vpool,
            tc.tile_pool(name="tmp", bufs=2) as tpool,
            tc.tile_pool(name="h1p", bufs=2) as h1pool,
            tc.tile_pool(name="h2p", bufs=2) as h2pool,
            tc.tile_pool(name="pu", bufs=2, space="PSUM") as pu,
            tc.tile_pool(name="pm", bufs=2, space="PSUM") as pm,
        ):
            # ---- constants: fp8 weights on the (otherwise idle) gpsimd
            # SWDGE queue; small bf16/f32 consts + the 1/counts broadcast on
            # the scalar HWDGE ring, leaving the sync ring free for bulk.
            wc8_sb = cpool.tile([128, HC + LC, TRANS], fp8, tag="wc8")
            nc.gpsimd.dma_start(wc8_sb[:], wc8_d.ap().rearrange("p (c m) -> p c m", m=TRANS))
            wc16_sb = cpool.tile([128, TRANS + 2], bf16, tag="wc16")
            nc.scalar.dma_start(wc16_sb[:], wc16_d[:, :])
            cf_sb = cpool.tile([TRANS, 5], f32, tag="cf")
            nc.scalar.dma_start(cf_sb[:], cf_d[:, :])
            cb_sb = cpool.tile([128, BS], bf16, tag="cb")
            nc.scalar.dma_start(cb_sb[:], cb_d[:, 0:BS].broadcast_to([128, BS]))

            # ---- bulk tile blobs on the sync HWDGE ring. Tile 0 is split
            # so its mask+text half lands ~1.1us earlier; the rest are one
            # ~1MB DMA each (near line rate).
            blob_sb = []
            b0 = bpool.tile([128, TILE_B], fp8, tag="blob")
            nc.sync.dma_start(b0[:, 0 : MT_B + XT_B], blob_d[:, 0, 0 : MT_B + XT_B])
            nc.sync.dma_start(b0[:, MT_B + XT_B :], blob_d[:, 0, MT_B + XT_B :])
            blob_sb.append(b0)
            for i in range(1, NT):
                bi = bpool.tile([128, TILE_B], fp8, tag="blob")
                nc.sync.dma_start(bi[:], blob_d[:, i, :])
                blob_sb.append(bi)

            def lw2p(c):   # fp8 mask-weight pair [128, 2, TRANS]
                return wc8_sb[:, HC + 2 * c : HC + 2 * c + 2, :]

            def atTp(c):   # fp8 text-weight pair
                return wc8_sb[:, 2 * c : 2 * c + 2, :]

            w1T = wc16_sb[:, 0:TRANS]
            w2c = wc16_sb[:, TRANS : TRANS + 1]
            c0b = cf_sb[:, 0:1]
            b1b = cf_sb[:, 1:2]
            nb2 = cf_sb[0:1, 2:3]   # -b2 (row 0 scalar for the [1,*] e rows)
            pb2 = cf_sb[0:1, 3:4]   # +b2
            ones_col = cf_sb[:, 4:5]

            def pv(t, off):  # DoubleRow pair view [128, 2, BT] at byte offset
                return blob_sb[t][:, off : off + 2 * BT].rearrange("p (n j) -> p j n", j=2)

            def mt_ap(t, c):
                return pv(t, c * 2 * BT)

            def xt_ap(t, c):
                return pv(t, MT_B + c * 2 * BT)

            def xn_ap(t, c):
                return pv(t, MT_B + XT_B + c * 2 * BT)

            # ---- PE pre-warm: dummy matmuls with no input deps keep the PE
            # HAM activity window busy while the first loads are in flight.
            warm_sb = cpool.tile([128, BT], bf16, tag="warm")
            nc.vector.memset(warm_sb[:, :], 0)
            wp = pu.tile([128, 2 * BT], f32, tag="u")
            for _ in range(N_WARM):
                nc.tensor.matmul(wp[:, 0:BT], warm_sb[:, :TRANS], warm_sb[:, :],
                                 start=True, stop=True)

            NSEG = 2 * NT                     # 8 softplus row segments
            esp_sb = cpool.tile([1, NSEG * BT], f32, tag="esp")
            EPK = (NSEG - 1) * BT // 128      # 28 packed cols for segments 0..6
            epk_sb = cpool.tile([128, EPK], f32, tag="epk")
            acc2_sb = cpool.tile([128, 1], f32, tag="acc2")
            lnj_sb = cpool.tile([1, BT], f32, tag="lnj")
            acc7_sb = cpool.tile([1, 1], f32, tag="acc7")
            res_sb = cpool.tile([1, 1], f32, tag="res")

            # ---- main loop: supertiles of 2 batch tiles ----
            for S in range(NT // 2):
                ta, tb = 2 * S, 2 * S + 1
                u = {}
                vsb = {}
                for t in (ta, tb):
                    u[t] = pu.tile([128, 2 * BT], f32, tag="u", name=f"u{t}")
                # v = (mask @ LW2).T into the joint-u bank (freed by the vs
                # mul before the text matmuls overwrite it)
                for c in range(LP):
                    for t in (ta, tb):
                        nc.tensor.matmul(u[t][:, 0:BT], lw2p(c), mt_ap(t, c),
                                         start=(c == 0), stop=(c == LP - 1),
                                         perf_mode=DR)
                for t in (ta, tb):
                    vt = vpool.tile([128, 1, BT], f32, tag="vs")
                    nc.vector.tensor_mul(vt[:, 0, :], u[t][:, 0:BT],
                                         cb_sb[:, t * BT : (t + 1) * BT])
                    vsb[t] = vt
                # text matmuls, weight-stationary: one LDWEIGHTS per pair
                # streams 4 matmuls (2 tiles x {joint, marginal})
                for c in range(HP):
                    for t in (ta, tb):
                        nc.tensor.matmul(u[t][:, 0:BT], atTp(c), xt_ap(t, c),
                                         start=(c == 0), stop=(c == HP - 1),
                                         perf_mode=DR)
                        nc.tensor.matmul(u[t][:, BT:], atTp(c), xn_ap(t, c),
                                         start=(c == 0), stop=(c == HP - 1),
                                         perf_mode=DR)
                # elementwise + head, per tile
                for t in (ta, tb):
                    tmp = tpool.tile([128, 2 * BT], f32, tag="tmp")
                    nc.vector.tensor_add(
                        tmp[:, :].rearrange("p (s n) -> p s n", s=2),
                        u[t][:, :].rearrange("p (s n) -> p s n", s=2),
                        vsb[t][:, :, :].broadcast_to([128, 2, BT]))
                    h1 = h1pool.tile([128, 2 * BT], bf16, tag="h1")
                    nc.vector.tensor_scalar(h1[:, :], tmp[:, :], c0b, 0.0,
                                            op0=ALU.add, op1=ALU.max)
                    hm = pm.tile([128, 2 * BT], f32, tag="hm")
                    nc.tensor.matmul(hm[:, 0:BT], w1T, h1[:, 0:BT], start=True, stop=True)
                    nc.tensor.matmul(hm[:, BT:], w1T, h1[:, BT:], start=True, stop=True)
                    h2s = h2pool.tile([128, 2 * BT], bf16, tag="h2s")
                    nc.scalar.activation(h2s[:, :], hm[:, :], AF.Relu, bias=b1b)
                    # e rows land in row 0 of the (drained) h2 banks
                    nc.tensor.matmul(hm[0:1, 0:BT], w2c, h2s[:, 0:BT], start=True, stop=True)
                    nc.tensor.matmul(hm[0:1, BT:], w2c, h2s[:, BT:], start=True, stop=True)
                    # exp with the +-b2 bias and the joint-stream negation
                    # folded in: joint exp(-(e+b2)), marginal exp(e+b2). The
                    # later ln(1+y) pass is then sign-agnostic, so the packed
                    # [128, 28] layout may mix segments freely.
                    sj = 2 * t
                    nc.scalar.activation(esp_sb[:, sj * BT : (sj + 1) * BT],
                                         hm[0:1, 0:BT], AF.Exp,
                                         bias=nb2, scale=-1.0)
                    if sj == NSEG - 2:
                        # segments 0..6 done after the joint exp of the last
                        # tile: repack them across partitions (overlaps with
                        # the remaining marginal chain)
                        nc.sync.dma_start(epk_sb[:, :], esp_sb[:, 0 : (NSEG - 1) * BT])
                    nc.scalar.activation(esp_sb[:, (sj + 1) * BT : (sj + 2) * BT],
                                         hm[0:1, BT:], AF.Exp, bias=pb2)

            # softplus tail: ln(1+y) over the packed block + the last row
            nc.scalar.activation(epk_sb[:, :], epk_sb[:, :], AF.Ln,
                                 bias=1.0, accum_out=acc2_sb[:, :])
            nc.scalar.activation(lnj_sb[:, :], esp_sb[:, (NSEG - 1) * BT :],
                                 AF.Ln, bias=1.0, accum_out=acc7_sb[:, :])
            res_ps = pm.tile([128, 2 * BT], f32, tag="hm")
            nc.tensor.matmul(res_ps[0:1, 0:1], acc2_sb[:, :], ones_col,
                             start=True, stop=True)
            nc.vector.tensor_add(res_sb[:, :], res_ps[0:1, 0:1], acc7_sb[:, :])
            nc.sync.dma_start(out_d[:, :], res_sb[:, :])

    _split_sync_waits(nc, mybir, maxw_default=1, maxw_drain=1)
    return nc


def _get_nc():
    if "nc" not in _CACHE:
        _CACHE["nc"] = _build()
    return _CACHE["nc"]


def _prep_inputs(text_embed, label_embed, target, perm,
                 W_text, b_text, W_label, b_label, W0, b0, W1, b1, W2, b2):
    f64 = np.float64
    W0t = W0[:, :TRANS].astype(f64)
    W0l = W0[:, TRANS:].astype(f64)
    A_t = W0t @ W_text.astype(f64)                                   # [T, HID]
    LW2 = (label_embed.astype(f64) @ W_label.T.astype(f64)) @ W0l.T  # [L, T]
    c0 = b0.astype(f64) + W0t @ b_text.astype(f64) + W0l @ b_label.astype(f64)

    atT_p = np.ascontiguousarray(A_t.T).reshape(HC, 128, TRANS).transpose(1, 0, 2).reshape(128, HID)
    lw2_p = np.ascontiguousarray(LW2).reshape(LC, 128, TRANS).transpose(1, 0, 2).reshape(128, L)
    wc8 = np.concatenate([atT_p, lw2_p], axis=1).astype(FP8)

    b2v = float(np.asarray(b2).reshape(-1)[0])
    wc16 = np.concatenate(
        [W1.T.astype(f64), W2.T.reshape(TRANS, 1).astype(f64), np.zeros((TRANS, 1))],
        axis=1).astype(BF16)                                         # [128, 130]
    cf = np.stack([c0, b1.astype(f64), np.full(TRANS, -b2v), np.full(TRANS, b2v),
                   np.ones(TRANS)], axis=1).astype(np.float32)       # [128, 5]

    counts = np.maximum(target.sum(axis=1), 1).astype(f64)
    cinv = (1.0 / counts).astype(BF16)                               # [B] bf16

    text_T = np.ascontiguousarray(text_embed.T).astype(FP8)          # [HID, B]
    mask_T = np.ascontiguousarray(target.T.astype(np.float32)).astype(FP8)  # [L, B]
    perm = np.asarray(perm).astype(np.int64)

    def interleave(a):
        # [2G*128, N] -> [128, G, 2N] fp8 with k-chunk pairs adjacent per column
        g2, n = a.shape[0] // 256, a.shape[1]
        return np.ascontiguousarray(
            a.reshape(g2, 2, 128, n).transpose(2, 0, 3, 1).reshape(128, g2, 2 * n)
        )

    in_maps = []
    for k in range(NCORES):
        sl = slice(k * BS, (k + 1) * BS)
        mtI = interleave(mask_T[:, sl])          # [128, LP, 2*BS]
        xtI = interleave(text_T[:, sl])          # [128, HP, 2*BS]
        xnI = interleave(text_T[:, perm[sl]])    # [128, HP, 2*BS]
        tiles = []
        for i in range(NT):
            sl2 = slice(2 * i * BT, 2 * (i + 1) * BT)
            tiles.append(np.concatenate(
                [mtI[:, :, sl2].reshape(128, -1),
                 xtI[:, :, sl2].reshape(128, -1),
                 xnI[:, :, sl2].reshape(128, -1)], axis=1))
        blob = np.ascontiguousarray(np.stack(tiles, axis=1))  # [128, NT, TILE_B]
        in_maps.append({
            "blob": blob,
            "wc8": wc8, "wc16": wc16, "cf": cf,
            "cbv": np.ascontiguousarray(cinv[sl]).reshape(1, BS),
        })
    return in_maps, b2v


def _run(in_maps, b2val, trace=False):
    from concourse.bass_utils import run_bass_kernel_spmd

    nc = _get_nc()
    res = run_bass_kernel_spmd(nc, in_maps, list(range(NCORES)), trace=trace)
    total = sum(float(res.results[k]["out"][0, 0]) for k in range(NCORES))
    return np.float32(total / B), res


def kernel(text_embed, label_embed, target, perm,
           W_text, b_text, W_label, b_label, W0, b0, W1, b1, W2, b2):
    in_maps, b2val = _prep_inputs(
        text_embed, label_embed, target, perm,
        W_text, b_text, W_label, b_label, W0, b0, W1, b1, W2, b2)
    out, _ = _run(in_maps, b2val)
    return out


# revision 19
# speedup vs baseline: 1.1107x; 1.1107x over previous
"""Trainium2 Bass kernel for the MINE-style segment_reduce problem.

Computes, for the fixed problem size B=16384, L=512, HID=768, TRANS=128:

    mask   = target.astype(f32)                     # [B, L] of {0,1}
    counts = max(mask.sum(1), 1)
    lf     = (mask @ label_embed) / counts          # [B, HID]
    net(t) = MLP(concat(t @ W_text.T + b_text, lf @ W_label.T + b_label))
    out    = mean(softplus(net(text[perm]))) + mean(softplus(-net(text)))

Algebraic folding (exact in real arithmetic): the first two linear layers
collapse into

    h1 = relu(text @ A_t.T + (mask @ LW2) / counts + c0)
    A_t = W0[:, :T] @ W_text                        # [T, HID]
    LW2 = (label_embed @ W_label.T) @ W0[:, T:].T   # [L, T]
    c0  = b0 + W0[:, :T] @ b_text + W0[:, T:] @ b_label

so label_embed never reaches the device; the per-sample network is two
small matmuls + relu + softplus.

Sharding: data-parallel over B across 8 NeuronCores (2048 rows each).
negative_text = text[perm] is realized host-side as a per-shard gather.
Each core returns the partial softplus sum over its rows; the host adds
8 scalars and divides by B.

Device-side design (v2):
 - All per-tile bulk data (mask | text | neg-text, fp8, pair-interleaved
   for DoubleRow) is packed host-side into ONE dram blob per batch tile,
   so the whole input streams in with 5 large HWDGE DMAs on the sync
   ring (HWDGE descriptor-gen is ~600ns of serial SP time per DMA - the
   v1 kernel spent ~17us there across 27 DMAs).
 - Weight-stationary matmul ordering: each DoubleRow fp8 weight pair is
   loaded once per 2-tile supertile and streams 4 matmuls (2 tiles x
   2 streams), instead of paying the 213ns LDWEIGHTS per matmul.
 - v = mask @ LW2 lands in the same PSUM bank that the joint-stream text
   matmuls later accumulate into (WAR handled by Tile), and the e rows
   land in the h2 banks, so the whole pipeline fits in 8 PSUM banks
   with 2-supertile double buffering.
 - 1/counts is applied as a bf16 row broadcast (one HWDGE DMA) and a DVE
   multiply; c0 and the relu fold into one DVE tensor_scalar; b1 folds
   into the h2 relu on ACT.
 - softplus runs directly on the [1,512] e-rows on ACT with accum_out,
   with +-b2 as the activation bias, so there is no staging copy, no
   repack DMA, and no final cross-partition matmul (v1 spent ~7us in
   that serialized tail).
"""

import numpy as np
import ml_dtypes

B, L, HID, TRANS = 16384, 512, 768, 128
NCORES = 8
BS = B // NCORES          # 2048 rows per core
BT = 512                  # batch tile (free-dim columns per PSUM bank)
NT = BS // BT             # 4 tiles per core
HC = HID // 128           # 6 contraction chunks for text
LC = L // 128             # 4 contraction chunks for the mask
HP = HC // 2              # 3 DoubleRow pairs for text
LP = LC // 2              # 2 DoubleRow pairs for the mask

MT_B = LP * 2 * BT        # 2048 mask bytes per partition per tile
XT_B = HP * 2 * BT        # 3072 text bytes per partition per tile
TILE_B = MT_B + 2 * XT_B  # 8192 = mask | neg-text | text

BF16 = ml_dtypes.bfloat16
FP8 = ml_dtypes.float8_e4m3

_CACHE = {}


def _split_sync_waits(nc, mybir, maxw_default=1, maxw_drain=1):
    """Walrus in this container rejects too many sync-waits per instruction
    ("Too many sync wait commands"). Hoist excess waits onto NoOps that
    precede the instruction on the same engine."""
    for f in nc.m.functions:
        for bb in f.blocks:
            new = []
            for inst in bb.instructions:
                maxw = maxw_drain if type(inst).__name__ in ("InstDrain", "InstNoOp") else maxw_default
                si = inst.sync_info
                if si is not None and si.on_wait is not None and len(si.on_wait) > maxw:
                    waits = list(si.on_wait)
                    head, rest = waits[:-maxw], waits[-maxw:]
                    for k in range(0, len(head), maxw_drain):
                        nop = mybir.InstNoOp(name=f"{inst.name}-w{k}", ins=[], outs=[])
                        nop.engine = inst.engine
                        nop.sync_info = mybir.SyncInfo(
                            on_wait=head[k : k + maxw_drain], on_update=[]
                        )
                        new.append(nop)
                    inst.sync_info = mybir.SyncInfo(
                        on_wait=rest, on_update=list(si.on_update or [])
                    )
                new.append(inst)
            bb.instructions = new


N_WARM = 4


def _build():
    import concourse.bass as bass
    import concourse.mybir as mybir
    import concourse.tile as tile

    f32 = mybir.dt.float32
    bf16 = mybir.dt.bfloat16
    fp8 = mybir.dt.float8e4

    nc = bass.Bass("TRN2", target_bir_lowering=False, debug=False, num_devices=NCORES)

    blob_d = nc.declare_dram_parameter("blob", [128, NT, TILE_B], fp8, isOutput=False)
    wc8_d = nc.declare_dram_parameter("wc8", [128, (HC + LC) * TRANS], fp8, isOutput=False)
    wc16_d = nc.declare_dram_parameter("wc16", [128, TRANS + 2], bf16, isOutput=False)
    cf_d = nc.declare_dram_parameter("cf", [TRANS, 5], f32, isOutput=False)
    cb_d = nc.declare_dram_parameter("cbv", [1, BS], bf16, isOutput=False)
    out_d = nc.declare_dram_parameter("out", [1, 1], f32, isOutput=True)

    AF = mybir.ActivationFunctionType
    ALU = mybir.AluOpType
    DR = mybir.MatmulPerfMode.DoubleRow

    with tile.TileContext(nc) as tc:
        with (
            tc.tile_pool(name="const", bufs=1) as cpool,
            tc.tile_pool(name="blob", bufs=NT) as bpool,
            tc.tile_pool(name="vs", bufs=2) as vpool,
            tc.tile_pool(name="tmp", bufs=2) as tpool,
            tc.tile_pool(name="h1p", bufs=2) as h1pool,
            tc.tile_pool(name="h2p", bufs=2) as h2pool,
            tc.tile_pool(name="pu", bufs=2, space="PSUM") as pu,
            tc.tile_pool(name="pm", bufs=2, space="PSUM") as pm,
        ):
            # ---- constants: the 1/counts broadcast goes FIRST on the scalar
            # HWDGE ring (it gates the vs multiplies, which gate the joint
            # text matmuls via the shared v/u PSUM bank); fp8 weights ride
            # the idle gpsimd SWDGE queue as one flat contiguous copy.
            cb_sb = cpool.tile([128, BS], bf16, tag="cb")
            nc.scalar.dma_start(cb_sb[:], cb_d[:, 0:BS].broadcast_to([128, BS]))
            wc8_sb = cpool.tile([128, HC + LC, TRANS], fp8, tag="wc8")
            nc.gpsimd.dma_start(wc8_sb[:, :, :].rearrange("p c m -> p (c m)"), wc8_d[:, :])
            wc16_sb = cpool.tile([128, TRANS + 2], bf16, tag="wc16")
            nc.scalar.dma_start(wc16_sb[:], wc16_d[:, :])
            cf_sb = cpool.tile([TRANS, 5], f32, tag="cf")
            nc.scalar.dma_start(cf_sb[:], cf_d[:, :])

            # ---- bulk tile blobs on the sync HWDGE ring, two pieces per
            # tile ([mask|neg-text] then [text]), interleaved per supertile
            # so both tiles' mask halves land before either text half.
            blob_sb = []
            for i in range(NT):
                bi = bpool.tile([128, TILE_B], fp8, tag="blob", name=f"b{i}")
                blob_sb.append(bi)
            for S in range(NT // 2):
                ta, tb = 2 * S, 2 * S + 1
                for t in (ta, tb):
                    nc.sync.dma_start(blob_sb[t][:, 0 : MT_B + XT_B],
                                      blob_d[:, t, 0 : MT_B + XT_B])
                for t in (ta, tb):
                    nc.sync.dma_start(blob_sb[t][:, MT_B + XT_B :],
                                      blob_d[:, t, MT_B + XT_B :])

            def lw2p(c):   # fp8 mask-weight pair [128, 2, TRANS]
                return wc8_sb[:, HC + 2 * c : HC + 2 * c + 2, :]

            def atTp(c):   # fp8 text-weight pair
                return wc8_sb[:, 2 * c : 2 * c + 2, :]

            w1T = wc16_sb[:, 0:TRANS]
            w2c = wc16_sb[:, TRANS : TRANS + 1]
            c0b = cf_sb[:, 0:1]
            b1b = cf_sb[:, 1:2]
            nb2 = cf_sb[0:1, 2:3]   # -b2 (row 0 scalar for the [1,*] e rows)
            pb2 = cf_sb[0:1, 3:4]   # +b2
            ones_col = cf_sb[:, 4:5]

            def pv(t, off):  # DoubleRow pair view [128, 2, BT] at byte offset
                return blob_sb[t][:, off : off + 2 * BT].rearrange("p (n j) -> p j n", j=2)

            def mt_ap(t, c):
                return pv(t, c * 2 * BT)

            def xn_ap(t, c):
                return pv(t, MT_B + c * 2 * BT)

            def xt_ap(t, c):
                return pv(t, MT_B + XT_B + c * 2 * BT)

            # ---- PE pre-warm: dummy matmuls with no input deps keep the PE
            # HAM activity window busy while the first loads are in flight.
            warm_sb = cpool.tile([128, BT], bf16, tag="warm")
            nc.vector.memset(warm_sb[:, :], 0)
            wp = pu.tile([128, 2 * BT], f32, tag="u")
            for _ in range(N_WARM):
                nc.tensor.matmul(wp[:, 0:BT], warm_sb[:, :TRANS], warm_sb[:, :],
                                 start=True, stop=True)

            NSEG = 2 * NT                     # 8 softplus row segments
            NPACK = NSEG - 2                  # segments 0..5 go the packed path
            esp_sb = cpool.tile([1, NSEG * BT], f32, tag="esp")
            EPK = NPACK * BT // 128           # 24 packed cols
            epk_sb = cpool.tile([128, EPK], f32, tag="epk")
            acc2_sb = cpool.tile([128, 1], f32, tag="acc2")
            lnj_sb = cpool.tile([1, BT], f32, tag="lnj")
            accr_sb = cpool.tile([1, 2], f32, tag="accr")
            res_sb = cpool.tile([1, 1], f32, tag="res")

            # ---- main loop: supertiles of 2 batch tiles ----
            for S in range(NT // 2):
                ta, tb = 2 * S, 2 * S + 1
                u = {}
                vsb = {}
                for t in (ta, tb):
                    u[t] = pu.tile([128, 2 * BT], f32, tag="u", name=f"u{t}")
                # v = (mask @ LW2).T into the joint-u bank (freed by the vs
                # mul before the text matmuls overwrite it)
                for c in range(LP):
                    for t in (ta, tb):
                        nc.tensor.matmul(u[t][:, 0:BT], lw2p(c), mt_ap(t, c),
                                         start=(c == 0), stop=(c == LP - 1),
                                         perf_mode=DR)
                for t in (ta, tb):
                    vt = vpool.tile([128, 1, BT], f32, tag="vs")
                    nc.vector.tensor_mul(vt[:, 0, :], u[t][:, 0:BT],
                                         cb_sb[:, t * BT : (t + 1) * BT])
                    vsb[t] = vt
                # text matmuls, weight-stationary: one LDWEIGHTS per pair
                # streams 4 matmuls (2 tiles x {marginal, joint}). Marginal
                # first: the joint matmuls overwrite the v bank and so must
                # wait for the vs multiply (WAR).
                for c in range(HP):
                    for t in (ta, tb):
                        nc.tensor.matmul(u[t][:, BT:], atTp(c), xn_ap(t, c),
                                         start=(c == 0), stop=(c == HP - 1),
                                         perf_mode=DR)
                    for t in (ta, tb):
                        nc.tensor.matmul(u[t][:, 0:BT], atTp(c), xt_ap(t, c),
                                         start=(c == 0), stop=(c == HP - 1),
                                         perf_mode=DR)
                # elementwise + head, per tile
                for t in (ta, tb):
                    tmp = tpool.tile([128, 2 * BT], f32, tag="tmp")
                    nc.vector.tensor_add(
                        tmp[:, :].rearrange("p (s n) -> p s n", s=2),
                        u[t][:, :].rearrange("p (s n) -> p s n", s=2),
                        vsb[t][:, :, :].broadcast_to([128, 2, BT]))
                    h1 = h1pool.tile([128, 2 * BT], bf16, tag="h1")
                    nc.vector.tensor_scalar(h1[:, :], tmp[:, :], c0b, 0.0,
                                            op0=ALU.add, op1=ALU.max)
                    hm = pm.tile([128, 2 * BT], f32, tag="hm")
                    nc.tensor.matmul(hm[:, 0:BT], w1T, h1[:, 0:BT], start=True, stop=True)
                    nc.tensor.matmul(hm[:, BT:], w1T, h1[:, BT:], start=True, stop=True)
                    h2s = h2pool.tile([128, 2 * BT], bf16, tag="h2s")
                    nc.scalar.activation(h2s[:, :], hm[:, :], AF.Relu, bias=b1b)
                    # e rows land in row 0 of the (drained) h2 banks
                    nc.tensor.matmul(hm[0:1, 0:BT], w2c, h2s[:, 0:BT], start=True, stop=True)
                    nc.tensor.matmul(hm[0:1, BT:], w2c, h2s[:, BT:], start=True, stop=True)
                    # exp with the +-b2 bias and the joint-stream negation
                    # folded in: joint exp(-(e+b2)), marginal exp(e+b2). The
                    # later ln(1+y) pass is then sign-agnostic, so the packed
                    # [128, 28] layout may mix segments freely.
                    sj = 2 * t
                    nc.scalar.activation(esp_sb[:, sj * BT : (sj + 1) * BT],
                                         hm[0:1, 0:BT], AF.Exp,
                                         bias=nb2, scale=-1.0)
                    nc.scalar.activation(esp_sb[:, (sj + 1) * BT : (sj + 2) * BT],
                                         hm[0:1, BT:], AF.Exp, bias=pb2)
                    if sj + 2 == NPACK:
                        # segments 0..5 complete: repack them across
                        # partitions; the [128, 24] ln runs while the last
                        # tile's chain is still in flight.
                        nc.sync.dma_start(epk_sb[:, :], esp_sb[:, 0 : NPACK * BT])
                        nc.scalar.activation(epk_sb[:, :], epk_sb[:, :], AF.Ln,
                                             bias=1.0, accum_out=acc2_sb[:, :])

            # softplus tail: ln(1+y) row passes for the last two segments
            for k in range(2):
                sj = NPACK + k
                nc.scalar.activation(lnj_sb[:, :],
                                     esp_sb[:, sj * BT : (sj + 1) * BT],
                                     AF.Ln, bias=1.0,
                                     accum_out=accr_sb[:, k : k + 1])
            res_ps = pm.tile([128, 2 * BT], f32, tag="hm")
            nc.tensor.matmul(res_ps[0:1, 0:1], acc2_sb[:, :], ones_col,
                             start=True, stop=True)
            nc.vector.tensor_add(res_sb[:, :], res_ps[0:1, 0:1], accr_sb[:, 0:1])
            nc.vector.tensor_add(res_sb[:, :], res_sb[:, :], accr_sb[:, 1:2])
            nc.sync.dma_start(out_d[:, :], res_sb[:, :])

    _split_sync_waits(nc, mybir, maxw_default=1, maxw_drain=1)
    return nc


def _get_nc():
    if "nc" not in _CACHE:
        _CACHE["nc"] = _build()
    return _CACHE["nc"]


def _prep_inputs(text_embed, label_embed, target, perm,
                 W_text, b_text, W_label, b_label, W0, b0, W1, b1, W2, b2):
    f64 = np.float64
    W0t = W0[:, :TRANS].astype(f64)
    W0l = W0[:, TRANS:].astype(f64)
    A_t = W0t @ W_text.astype(f64)                                   # [T, HID]
    LW2 = (label_embed.astype(f64) @ W_label.T.astype(f64)) @ W0l.T  # [L, T]
    c0 = b0.astype(f64) + W0t @ b_text.astype(f64) + W0l @ b_label.astype(f64)

    atT_p = np.ascontiguousarray(A_t.T).reshape(HC, 128, TRANS).transpose(1, 0, 2).reshape(128, HID)
    lw2_p = np.ascontiguousarray(LW2).reshape(LC, 128, TRANS).transpose(1, 0, 2).reshape(128, L)
    wc8 = np.concatenate([atT_p, lw2_p], axis=1).astype(FP8)

    b2v = float(np.asarray(b2).reshape(-1)[0])
    wc16 = np.concatenate(
        [W1.T.astype(f64), W2.T.reshape(TRANS, 1).astype(f64), np.zeros((TRANS, 1))],
        axis=1).astype(BF16)                                         # [128, 130]
    cf = np.stack([c0, b1.astype(f64), np.full(TRANS, -b2v), np.full(TRANS, b2v),
                   np.ones(TRANS)], axis=1).astype(np.float32)       # [128, 5]

    counts = np.maximum(target.sum(axis=1), 1).astype(f64)
    cinv = (1.0 / counts).astype(BF16)                               # [B] bf16

    text_T = np.ascontiguousarray(text_embed.T).astype(FP8)          # [HID, B]
    mask_T = np.ascontiguousarray(target.T.astype(np.float32)).astype(FP8)  # [L, B]
    perm = np.asarray(perm).astype(np.int64)

    def interleave(a):
        # [2G*128, N] -> [128, G, 2N] fp8 with k-chunk pairs adjacent per column
        g2, n = a.shape[0] // 256, a.shape[1]
        return np.ascontiguousarray(
            a.reshape(g2, 2, 128, n).transpose(2, 0, 3, 1).reshape(128, g2, 2 * n)
        )

    in_maps = []
    for k in range(NCORES):
        sl = slice(k * BS, (k + 1) * BS)
        mtI = interleave(mask_T[:, sl])          # [128, LP, 2*BS]
        xtI = interleave(text_T[:, sl])          # [128, HP, 2*BS]
        xnI = interleave(text_T[:, perm[sl]])    # [128, HP, 2*BS]
        tiles = []
        for i in range(NT):
            sl2 = slice(2 * i * BT, 2 * (i + 1) * BT)
            tiles.append(np.concatenate(
                [mtI[:, :, sl2].reshape(128, -1),
                 xnI[:, :, sl2].reshape(128, -1),
                 xtI[:, :, sl2].reshape(128, -1)], axis=1))
        blob = np.ascontiguousarray(np.stack(tiles, axis=1))  # [128, NT, TILE_B]
        in_maps.append({
            "blob": blob,
            "wc8": wc8, "wc16": wc16, "cf": cf,
            "cbv": np.ascontiguousarray(cinv[sl]).reshape(1, BS),
        })
    return in_maps, b2v


def _run(in_maps, b2val, trace=False):
    from concourse.bass_utils import run_bass_kernel_spmd

    nc = _get_nc()
    res = run_bass_kernel_spmd(nc, in_maps, list(range(NCORES)), trace=trace)
    total = sum(float(res.results[k]["out"][0, 0]) for k in range(NCORES))
    return np.float32(total / B), res


def kernel(text_embed, label_embed, target, perm,
           W_text, b_text, W_label, b_label, W0, b0, W1, b1, W2, b2):
    in_maps, b2val = _prep_inputs(
        text_embed, label_embed, target, perm,
        W_text, b_text, W_label, b_label, W0, b0, W1, b1, W2, b2)
    out, _ = _run(in_maps, b2val)
    return out


# revision 26
# speedup vs baseline: 1.1566x; 1.0414x over previous
"""Trainium2 Bass kernel for the MINE-style segment_reduce problem.

Computes, for the fixed problem size B=16384, L=512, HID=768, TRANS=128:

    mask   = target.astype(f32)                     # [B, L] of {0,1}
    counts = max(mask.sum(1), 1)
    lf     = (mask @ label_embed) / counts          # [B, HID]
    net(t) = MLP(concat(t @ W_text.T + b_text, lf @ W_label.T + b_label))
    out    = mean(softplus(net(text[perm]))) + mean(softplus(-net(text)))

Algebraic folding (exact in real arithmetic): the first two linear layers
collapse into

    h1 = relu(text @ A_t.T + (mask @ LW2) / counts + c0)
    A_t = W0[:, :T] @ W_text                        # [T, HID]
    LW2 = (label_embed @ W_label.T) @ W0[:, T:].T   # [L, T]
    c0  = b0 + W0[:, :T] @ b_text + W0[:, T:] @ b_label

so label_embed never reaches the device; the per-sample network is two
small matmuls + relu + softplus.

Sharding: data-parallel over B across 8 NeuronCores (2048 rows each).
negative_text = text[perm] is realized host-side as a per-shard gather.
Each core returns the partial softplus sum over its rows; the host adds
8 scalars and divides by B.

Device-side design (v2):
 - All per-tile bulk data (mask | text | neg-text, fp8, pair-interleaved
   for DoubleRow) is packed host-side into ONE dram blob per batch tile,
   so the whole input streams in with 5 large HWDGE DMAs on the sync
   ring (HWDGE descriptor-gen is ~600ns of serial SP time per DMA - the
   v1 kernel spent ~17us there across 27 DMAs).
 - Weight-stationary matmul ordering: each DoubleRow fp8 weight pair is
   loaded once per 2-tile supertile and streams 4 matmuls (2 tiles x
   2 streams), instead of paying the 213ns LDWEIGHTS per matmul.
 - v = mask @ LW2 lands in the same PSUM bank that the joint-stream text
   matmuls later accumulate into (WAR handled by Tile), and the e rows
   land in the h2 banks, so the whole pipeline fits in 8 PSUM banks
   with 2-supertile double buffering.
 - 1/counts is applied as a bf16 row broadcast (one HWDGE DMA) and a DVE
   multiply; c0 and the relu fold into one DVE tensor_scalar; b1 folds
   into the h2 relu on ACT.
 - softplus runs directly on the [1,512] e-rows on ACT with accum_out,
   with +-b2 as the activation bias, so there is no staging copy, no
   repack DMA, and no final cross-partition matmul (v1 spent ~7us in
   that serialized tail).
"""

import numpy as np
import ml_dtypes

B, L, HID, TRANS = 16384, 512, 768, 128
NCORES = 8
BS = B // NCORES          # 2048 rows per core
BT = 512                  # batch tile (free-dim columns per PSUM bank)
NT = BS // BT             # 4 tiles per core
HC = HID // 128           # 6 contraction chunks for text
LC = L // 128             # 4 contraction chunks for the mask
HP = HC // 2              # 3 DoubleRow pairs for text
LP = LC // 2              # 2 DoubleRow pairs for the mask

MT_B = LP * 2 * BT        # 2048 mask bytes per partition per tile
XT_B = HP * 2 * BT        # 3072 text bytes per partition per tile
TILE_B = MT_B + 2 * XT_B  # 8192 = mask | neg-text | text

BF16 = ml_dtypes.bfloat16
FP8 = ml_dtypes.float8_e4m3

_CACHE = {}


def _split_sync_waits(nc, mybir, maxw_default=1, maxw_drain=1):
    """Walrus in this container rejects too many sync-waits per instruction
    ("Too many sync wait commands"). Hoist excess waits onto NoOps that
    precede the instruction on the same engine."""
    for f in nc.m.functions:
        for bb in f.blocks:
            new = []
            for inst in bb.instructions:
                maxw = maxw_drain if type(inst).__name__ in ("InstDrain", "InstNoOp") else maxw_default
                si = inst.sync_info
                if si is not None and si.on_wait is not None and len(si.on_wait) > maxw:
                    waits = list(si.on_wait)
                    head, rest = waits[:-maxw], waits[-maxw:]
                    for k in range(0, len(head), maxw_drain):
                        nop = mybir.InstNoOp(name=f"{inst.name}-w{k}", ins=[], outs=[])
                        nop.engine = inst.engine
                        nop.sync_info = mybir.SyncInfo(
                            on_wait=head[k : k + maxw_drain], on_update=[]
                        )
                        new.append(nop)
                    inst.sync_info = mybir.SyncInfo(
                        on_wait=rest, on_update=list(si.on_update or [])
                    )
                new.append(inst)
            bb.instructions = new


N_WARM = 6


def _build():
    import concourse.bass as bass
    import concourse.mybir as mybir
    import concourse.tile as tile

    f32 = mybir.dt.float32
    bf16 = mybir.dt.bfloat16
    fp8 = mybir.dt.float8e4

    nc = bass.Bass("TRN2", target_bir_lowering=False, debug=False, num_devices=NCORES)

    blob_d = nc.declare_dram_parameter("blob", [128, NT, TILE_B], fp8, isOutput=False)
    wc8_d = nc.declare_dram_parameter("wc8", [128, (HC + LC) * TRANS], fp8, isOutput=False)
    wc16_d = nc.declare_dram_parameter("wc16", [128, TRANS + 2], bf16, isOutput=False)
    cf_d = nc.declare_dram_parameter("cf", [TRANS, 5], f32, isOutput=False)
    cb_d = nc.declare_dram_parameter("cbv", [128, BS], bf16, isOutput=False)
    out_d = nc.declare_dram_parameter("out", [1, 1], f32, isOutput=True)

    AF = mybir.ActivationFunctionType
    ALU = mybir.AluOpType
    DR = mybir.MatmulPerfMode.DoubleRow

    with tile.TileContext(nc) as tc:
        with (
            tc.tile_pool(name="const", bufs=1) as cpool,
            tc.tile_pool(name="blob", bufs=NT) as bpool,
            tc.tile_pool(name="vs", bufs=2) as vpool,
            tc.tile_pool(name="tmp", bufs=2) as tpool,
            tc.tile_pool(name="h1p", bufs=2) as h1pool,
            tc.tile_pool(name="h2p", bufs=2) as h2pool,
            tc.tile_pool(name="pu", bufs=2, space="PSUM") as pu,
            tc.tile_pool(name="pm", bufs=2, space="PSUM") as pm,
        ):
            # ---- constants, all on the scalar HWDGE ring (the gpsimd SWDGE
            # Q7 path adds ~4us of descriptor-emission lag). Order matters:
            # wc8 gates the first mask matmul; the 1/counts broadcast gates
            # the vs multiplies (and through the shared v/u PSUM bank, the
            # joint text matmuls). The broadcast is done SBUF->SBUF (load
            # the 4KB row first) - the HBM step-0 spray measured ~8us.
            wc8_sb = cpool.tile([128, HC + LC, TRANS], fp8, tag="wc8")
            nc.scalar.dma_start(wc8_sb[:, :, :].rearrange("p c m -> p (c m)"), wc8_d[:, :])
            cb_sb = cpool.tile([128, BS], bf16, tag="cb")
            nc.scalar.dma_start(cb_sb[:], cb_d[:, :])
            wc16_sb = cpool.tile([128, TRANS + 2], bf16, tag="wc16")
            nc.scalar.dma_start(wc16_sb[:], wc16_d[:, :])
            cf_sb = cpool.tile([TRANS, 5], f32, tag="cf")
            nc.scalar.dma_start(cf_sb[:], cf_d[:, :])

            # ---- bulk tile blobs on the sync HWDGE ring, two pieces per
            # tile ([mask|neg-text] then [text]), interleaved per supertile
            # so both tiles' mask halves land before either text half.
            blob_sb = []
            for i in range(NT):
                bi = bpool.tile([128, TILE_B], fp8, tag="blob", name=f"b{i}")
                blob_sb.append(bi)
            for S in range(NT // 2):
                ta, tb = 2 * S, 2 * S + 1
                for t in (ta, tb):
                    nc.sync.dma_start(blob_sb[t][:, 0 : MT_B + XT_B],
                                      blob_d[:, t, 0 : MT_B + XT_B])
                for t in (ta, tb):
                    nc.sync.dma_start(blob_sb[t][:, MT_B + XT_B :],
                                      blob_d[:, t, MT_B + XT_B :])

            def lw2p(c):   # fp8 mask-weight pair [128, 2, TRANS]
                return wc8_sb[:, HC + 2 * c : HC + 2 * c + 2, :]

            def atTp(c):   # fp8 text-weight pair
                return wc8_sb[:, 2 * c : 2 * c + 2, :]

            w1T = wc16_sb[:, 0:TRANS]
            w2c = wc16_sb[:, TRANS : TRANS + 1]
            c0b = cf_sb[:, 0:1]
            b1b = cf_sb[:, 1:2]
            nb2 = cf_sb[0:1, 2:3]   # -b2 (row 0 scalar for the [1,*] e rows)
            pb2 = cf_sb[0:1, 3:4]   # +b2
            ones_col = cf_sb[:, 4:5]

            def pv(t, off):  # DoubleRow pair view [128, 2, BT] at byte offset
                return blob_sb[t][:, off : off + 2 * BT].rearrange("p (n j) -> p j n", j=2)

            def mt_ap(t, c):
                return pv(t, c * 2 * BT)

            def xn_ap(t, c):
                return pv(t, MT_B + c * 2 * BT)

            def xt_ap(t, c):
                return pv(t, MT_B + XT_B + c * 2 * BT)

            # ---- PE pre-warm: dummy matmuls with no input deps keep the PE
            # HAM activity window busy while the first loads are in flight.
            warm_sb = cpool.tile([128, BT], bf16, tag="warm")
            nc.vector.memset(warm_sb[:, :], 0)
            wp = pu.tile([128, 2 * BT], f32, tag="u")
            for _ in range(N_WARM):
                nc.tensor.matmul(wp[:, 0:BT], warm_sb[:, :TRANS], warm_sb[:, :],
                                 start=True, stop=True)

            NSEG = 2 * NT                     # 8 softplus row segments
            NPACK = NSEG - 2                  # segments 0..5 go the packed path
            esp_sb = cpool.tile([1, NSEG * BT], f32, tag="esp")
            EPK = NPACK * BT // 128           # 24 packed cols
            epk_sb = cpool.tile([128, EPK], f32, tag="epk")
            acc2_sb = cpool.tile([128, 1], f32, tag="acc2")
            lnj_sb = cpool.tile([1, 2 * BT], f32, tag="lnj")
            accr_sb = cpool.tile([1, 2], f32, tag="accr")
            res_sb = cpool.tile([1, 1], f32, tag="res")

            # ---- main loop: supertiles of 2 batch tiles ----
            for S in range(NT // 2):
                ta, tb = 2 * S, 2 * S + 1
                u = {}
                vsb = {}
                for t in (ta, tb):
                    u[t] = pu.tile([128, 2 * BT], f32, tag="u", name=f"u{t}")
                # v = (mask @ LW2).T into the joint-u bank (freed by the vs
                # mul before the text matmuls overwrite it)
                for c in range(LP):
                    for t in (ta, tb):
                        nc.tensor.matmul(u[t][:, 0:BT], lw2p(c), mt_ap(t, c),
                                         start=(c == 0), stop=(c == LP - 1),
                                         perf_mode=DR)
                for t in (ta, tb):
                    vt = vpool.tile([128, 1, BT], f32, tag="vs")
                    nc.vector.tensor_mul(vt[:, 0, :], u[t][:, 0:BT],
                                         cb_sb[:, t * BT : (t + 1) * BT])
                    vsb[t] = vt
                # text matmuls, weight-stationary: one LDWEIGHTS per pair
                # streams 4 matmuls (2 tiles x {marginal, joint}). Marginal
                # first: the joint matmuls overwrite the v bank and so must
                # wait for the vs multiply (WAR).
                for c in range(HP):
                    for t in (ta, tb):
                        nc.tensor.matmul(u[t][:, BT:], atTp(c), xn_ap(t, c),
                                         start=(c == 0), stop=(c == HP - 1),
                                         perf_mode=DR)
                    for t in (ta, tb):
                        nc.tensor.matmul(u[t][:, 0:BT], atTp(c), xt_ap(t, c),
                                         start=(c == 0), stop=(c == HP - 1),
                                         perf_mode=DR)
                # elementwise + head, per tile. The first supertile uses
                # fused [128, 1024] ops (fewer instructions); the second
                # runs each stream's chain separately so the pipeline drain
                # after the last DMA is half as deep. exp folds in the +-b2
                # bias and the joint-stream negation (joint exp(-(e+b2)),
                # marginal exp(e+b2)), so the later ln(1+y) passes are
                # sign-agnostic and the packed layout may mix segments.
                for t in (ta, tb):
                    tmp = tpool.tile([128, 2 * BT], f32, tag="tmp")
                    h1 = h1pool.tile([128, 2 * BT], bf16, tag="h1")
                    hm = pm.tile([128, 2 * BT], f32, tag="hm")
                    h2s = h2pool.tile([128, 2 * BT], bf16, tag="h2s")
                    sj = 2 * t
                    if S == 0:
                        nc.vector.tensor_add(
                            tmp[:, :].rearrange("p (s n) -> p s n", s=2),
                            u[t][:, :].rearrange("p (s n) -> p s n", s=2),
                            vsb[t][:, :, :].broadcast_to([128, 2, BT]))
                        nc.vector.tensor_scalar(h1[:, :], tmp[:, :], c0b, 0.0,
                                                op0=ALU.add, op1=ALU.max)
                        nc.tensor.matmul(hm[:, 0:BT], w1T, h1[:, 0:BT], start=True, stop=True)
                        nc.tensor.matmul(hm[:, BT:], w1T, h1[:, BT:], start=True, stop=True)
                        nc.scalar.activation(h2s[:, :], hm[:, :], AF.Relu, bias=b1b)
                        # e rows land in row 0 of the (drained) h2 banks
                        nc.tensor.matmul(hm[0:1, 0:BT], w2c, h2s[:, 0:BT], start=True, stop=True)
                        nc.tensor.matmul(hm[0:1, BT:], w2c, h2s[:, BT:], start=True, stop=True)
                        nc.scalar.activation(esp_sb[:, sj * BT : (sj + 1) * BT],
                                             hm[0:1, 0:BT], AF.Exp,
                                             bias=nb2, scale=-1.0)
                        nc.scalar.activation(esp_sb[:, (sj + 1) * BT : (sj + 2) * BT],
                                             hm[0:1, BT:], AF.Exp, bias=pb2)
                    else:
                        for s, (usl, bias, scale) in enumerate(
                                ((slice(0, BT), nb2, -1.0),
                                 (slice(BT, 2 * BT), pb2, 1.0))):
                            nc.vector.tensor_add(tmp[:, usl], u[t][:, usl],
                                                 vsb[t][:, 0, :])
                            nc.vector.tensor_scalar(h1[:, usl], tmp[:, usl],
                                                    c0b, 0.0,
                                                    op0=ALU.add, op1=ALU.max)
                            nc.tensor.matmul(hm[:, usl], w1T, h1[:, usl],
                                             start=True, stop=True)
                            nc.scalar.activation(h2s[:, usl], hm[:, usl],
                                                 AF.Relu, bias=b1b)
                            nc.tensor.matmul(hm[0:1, usl], w2c, h2s[:, usl],
                                             start=True, stop=True)
                            nc.scalar.activation(
                                esp_sb[:, (sj + s) * BT : (sj + s + 1) * BT],
                                hm[0:1, usl], AF.Exp, bias=bias, scale=scale)
                            if sj + s + 1 == NPACK:
                                # segments 0..5 complete: repack across
                                # partitions; the [128, 24] ln overlaps the
                                # last tile's chain.
                                nc.sync.dma_start(epk_sb[:, :],
                                                  esp_sb[:, 0 : NPACK * BT])
                                nc.scalar.activation(epk_sb[:, :], epk_sb[:, :],
                                                     AF.Ln, bias=1.0,
                                                     accum_out=acc2_sb[:, :])

            # softplus tail: one ln(1+y) row pass over the last two segments
            nc.scalar.activation(lnj_sb[:, :],
                                 esp_sb[:, NPACK * BT : NSEG * BT],
                                 AF.Ln, bias=1.0, accum_out=accr_sb[:, 0:1])
            res_ps = pm.tile([128, 2 * BT], f32, tag="hm")
            nc.tensor.matmul(res_ps[0:1, 0:1], acc2_sb[:, :], ones_col,
                             start=True, stop=True)
            nc.vector.tensor_add(res_sb[:, :], res_ps[0:1, 0:1], accr_sb[:, 0:1])
            nc.sync.dma_start(out_d[:, :], res_sb[:, :])

    _split_sync_waits(nc, mybir, maxw_default=1, maxw_drain=1)
    return nc


def _get_nc():
    if "nc" not in _CACHE:
        _CACHE["nc"] = _build()
    return _CACHE["nc"]


def _prep_inputs(text_embed, label_embed, target, perm,
                 W_text, b_text, W_label, b_label, W0, b0, W1, b1, W2, b2):
    f64 = np.float64
    W0t = W0[:, :TRANS].astype(f64)
    W0l = W0[:, TRANS:].astype(f64)
    A_t = W0t @ W_text.astype(f64)                                   # [T, HID]
    LW2 = (label_embed.astype(f64) @ W_label.T.astype(f64)) @ W0l.T  # [L, T]
    c0 = b0.astype(f64) + W0t @ b_text.astype(f64) + W0l @ b_label.astype(f64)

    atT_p = np.ascontiguousarray(A_t.T).reshape(HC, 128, TRANS).transpose(1, 0, 2).reshape(128, HID)
    lw2_p = np.ascontiguousarray(LW2).reshape(LC, 128, TRANS).transpose(1, 0, 2).reshape(128, L)
    wc8 = np.concatenate([atT_p, lw2_p], axis=1).astype(FP8)

    b2v = float(np.asarray(b2).reshape(-1)[0])
    wc16 = np.concatenate(
        [W1.T.astype(f64), W2.T.reshape(TRANS, 1).astype(f64), np.zeros((TRANS, 1))],
        axis=1).astype(BF16)                                         # [128, 130]
    cf = np.stack([c0, b1.astype(f64), np.full(TRANS, -b2v), np.full(TRANS, b2v),
                   np.ones(TRANS)], axis=1).astype(np.float32)       # [128, 5]

    counts = np.maximum(target.sum(axis=1), 1).astype(f64)
    cinv = (1.0 / counts).astype(BF16)                               # [B] bf16

    text_T = np.ascontiguousarray(text_embed.T).astype(FP8)          # [HID, B]
    mask_T = np.ascontiguousarray(target.T.astype(np.float32)).astype(FP8)  # [L, B]
    perm = np.asarray(perm).astype(np.int64)

    def interleave(a):
        # [2G*128, N] -> [128, G, 2N] fp8 with k-chunk pairs adjacent per column
        g2, n = a.shape[0] // 256, a.shape[1]
        return np.ascontiguousarray(
            a.reshape(g2, 2, 128, n).transpose(2, 0, 3, 1).reshape(128, g2, 2 * n)
        )

    in_maps = []
    for k in range(NCORES):
        sl = slice(k * BS, (k + 1) * BS)
        mtI = interleave(mask_T[:, sl])          # [128, LP, 2*BS]
        xtI = interleave(text_T[:, sl])          # [128, HP, 2*BS]
        xnI = interleave(text_T[:, perm[sl]])    # [128, HP, 2*BS]
        tiles = []
        for i in range(NT):
            sl2 = slice(2 * i * BT, 2 * (i + 1) * BT)
            tiles.append(np.concatenate(
                [mtI[:, :, sl2].reshape(128, -1),
                 xnI[:, :, sl2].reshape(128, -1),
                 xtI[:, :, sl2].reshape(128, -1)], axis=1))
        blob = np.ascontiguousarray(np.stack(tiles, axis=1))  # [128, NT, TILE_B]
        in_maps.append({
            "blob": blob,
            "wc8": wc8, "wc16": wc16, "cf": cf,
            "cbv": np.ascontiguousarray(
                np.broadcast_to(cinv[sl].reshape(1, BS), (128, BS))),
        })
    return in_maps, b2v


def _run(in_maps, b2val, trace=False):
    from concourse.bass_utils import run_bass_kernel_spmd

    nc = _get_nc()
    res = run_bass_kernel_spmd(nc, in_maps, list(range(NCORES)), trace=trace)
    total = sum(float(res.results[k]["out"][0, 0]) for k in range(NCORES))
    return np.float32(total / B), res


def kernel(text_embed, label_embed, target, perm,
           W_text, b_text, W_label, b_label, W0, b0, W1, b1, W2, b2):
    in_maps, b2val = _prep_inputs(
        text_embed, label_embed, target, perm,
        W_text, b_text, W_label, b_label, W0, b0, W1, b1, W2, b2)
    out, _ = _run(in_maps, b2val)
    return out
